# revision 1
# baseline (speedup 1.0000x reference)
"""Causal self-attention (B=2, T=2048, D_in=1152, D=1024, H=16) on 8 trn2 cores.

Sharding: 2-way data parallel over batch x 4-way tensor parallel over heads.
Core c handles batch b = c//4 and heads [4g, 4g+4) with g = c%4.

Per-core dataflow (all matmuls in float32r, ~fp32 precision at bf16 speed):
  QT = (Wq_g)^T @ xp[b]^T   -> [256, 2048]   (head dims on partitions)
  KT likewise; V = xp[b] @ Wv_g in natural [T, 256] layout (T on partitions),
  stored with a ones-column per head: Vh~ = [V_h | 1] as [128, 16, 4, 65].
  Scores transposed: ST[k, q] = K Q^T built per 128-row k-block so softmax
  denominators come free: OT~ = Vh~^T @ exp(ST/8) accumulates [65, 512] in
  PSUM where row 64 is the softmax row-sum. exp is unshifted (scores are
  N(0,1) after scaling - safe in fp32). Causal mask = 0/1 multiply after exp
  on diagonal blocks only; fully-masked blocks are skipped.
  Normalize via reciprocal + rank-1 (K=32 zero-padded) broadcast matmul,
  write into OT_all [256, 2048], then Y_partial = OT_all^T @ Wp_g.
Host sums the 4 partial Y per batch and adds bp.
"""

import numpy as np

import concourse.bass as bass
import concourse.mybir as mybir
import concourse.tile as tile
from concourse import bacc
from concourse.bass_utils import run_bass_kernel_spmd

F32 = mybir.dt.float32
F32R = mybir.dt.float32r
AF = mybir.ActivationFunctionType
MUL = mybir.AluOpType.mult

B, T, DIN, D, H = 2, 2048, 1152, 1024, 16
HD = D // H           # 64 head dim
HLOC = 4              # heads per core
DLOC = HLOC * HD      # 256 local model dims
KC = DIN // 128       # 9 contraction chunks for projections
NT = T // 512         # 4 column tiles of 512
QC = T // 128         # 16 row chunks of 128
SCALE = 1.0 / np.sqrt(np.float32(HD))

_CACHE = {}


def _build():
    nc = bacc.Bacc(None)

    xpt = nc.dram_tensor("xpt", [DIN, T], F32, kind="ExternalInput")
    wq = nc.dram_tensor("wq", [DIN, DLOC], F32, kind="ExternalInput")
    wk = nc.dram_tensor("wk", [DIN, DLOC], F32, kind="ExternalInput")
    wv = nc.dram_tensor("wv", [DIN, DLOC], F32, kind="ExternalInput")
    bq = nc.dram_tensor("bq", [DLOC], F32, kind="ExternalInput")
    bk = nc.dram_tensor("bk", [DLOC], F32, kind="ExternalInput")
    bvp = nc.dram_tensor("bvp", [32, DLOC], F32, kind="ExternalInput")
    wp = nc.dram_tensor("wp", [DLOC, D], F32, kind="ExternalInput")
    mask = nc.dram_tensor("mask", [128, 4, 1024], F32, kind="ExternalInput")
    ones = nc.dram_tensor("ones", [32, 128], F32, kind="ExternalInput")
    onesm = nc.dram_tensor("onesm", [128, 4, 64], F32, kind="ExternalInput")
    onesr = nc.dram_tensor("onesr", [128, 512], F32, kind="ExternalInput")
    y = nc.dram_tensor("y", [T, D], F32, kind="ExternalOutput")

    xpt_r = xpt.rearrange("(ko p) t -> p ko t", p=128)
    wq_r = wq.rearrange("(ko p) d -> p ko d", p=128)
    wk_r = wk.rearrange("(ko p) d -> p ko d", p=128)
    wv_r = wv.rearrange("(ko p) d -> p ko d", p=128)
    wp_r = wp.rearrange("(c p) n -> p c n", p=128)
    bq_r = bq.rearrange("(m p) -> p m", p=128)
    bk_r = bk.rearrange("(m p) -> p m", p=128)

    with tile.TileContext(nc) as tc:
        with (
            tc.tile_pool(name="const", bufs=1) as cpool,
            tc.tile_pool(name="work", bufs=2) as wpool,
            tc.tile_pool(name="exp", bufs=3) as epool,
            tc.tile_pool(name="stg", bufs=5) as spool,
            tc.tile_pool(name="psB", bufs=1, space="PSUM") as psB,
            tc.tile_pool(name="psC", bufs=2, space="PSUM") as psC,
            tc.tile_pool(name="psX", bufs=2, space="PSUM") as psX,
            nc.allow_low_precision(reason="float32r matmul pipeline"),
        ):
            t_wq = cpool.tile([128, KC, DLOC], F32R, tag="t_wq")
            t_wk = cpool.tile([128, KC, DLOC], F32R, tag="t_wk")
            t_wv = cpool.tile([128, KC, DLOC], F32R, tag="t_wv")
            t_wp = cpool.tile([128, 2, D], F32R, tag="t_wp")
            t_mask = cpool.tile([128, 4, 1024], F32R, tag="t_mask")
            t_bq = cpool.tile([128, 2], F32, tag="t_bq")
            t_bk = cpool.tile([128, 2], F32, tag="t_bk")
            t_ones = cpool.tile([32, 128], F32R, tag="t_ones")
            t_bvp = cpool.tile([32, DLOC], F32R, tag="t_bvp")
            t_qt = cpool.tile([128, 2, T], F32R, tag="t_qt")
            t_kt = cpool.tile([128, 2, T], F32R, tag="t_kt")
            t_v = cpool.tile([128, QC, HLOC, HD + 1], F32R, tag="t_v")
            t_ot = cpool.tile([128, 2, T], F32R, tag="t_ot")
            t_onesm = cpool.tile([128, 4, 64], F32R, tag="t_onesm")
            t_rec4 = cpool.tile([128, 512], F32R, tag="t_rec4")
            t_sums4 = cpool.tile([128, 512], F32, tag="t_sums4")

            t_vones = cpool.tile([128, HLOC], F32R, tag="t_vones")

            # critical-path DMAs first: wq + first xp tile feed matmul #0
            nc.sync.dma_start(t_wq[:], wq_r.bitcast(F32R))
            t_xp0 = wpool.tile([128, KC, 512], F32R, tag="t_xp")
            nc.sync.dma_start(t_xp0[:], xpt_r[:, :, 0:512].bitcast(F32R))
            nc.sync.dma_start(t_wk[:], wk_r.bitcast(F32R))
            nc.sync.dma_start(t_wv[:], wv_r.bitcast(F32R))
            nc.sync.dma_start(t_bq[:], bq_r)
            nc.sync.dma_start(t_bk[:], bk_r)
            nc.sync.dma_start(t_ones[:], ones[:].bitcast(F32R))
            nc.sync.dma_start(t_bvp[:], bvp[:].bitcast(F32R))
            nc.sync.dma_start(t_vones[:], onesr[:, 0:HLOC].bitcast(F32R))


            def proj(nt, t_xp=None):
                c0 = 512 * nt
                if t_xp is None:
                    t_xp = wpool.tile([128, KC, 512], F32R, tag="t_xp")
                    nc.sync.dma_start(
                        t_xp[:], xpt_r[:, :, c0 : c0 + 512].bitcast(F32R)
                    )
                groups = []
                live = {}

                def qk_half(t_w, t_b, t_dst, m, half, t_xp=t_xp, c0=c0):
                    if half == 0:
                        p = psX.tile([128, 512], F32, tag="aux")
                        live[(id(t_w), m)] = p
                        ks = range(0, KC // 2)
                    else:
                        p = live.pop((id(t_w), m))
                        ks = range(KC // 2, KC)
                    for k in ks:
                        nc.tensor.matmul(
                            p[:],
                            t_w[:, k, 128 * m : 128 * m + 128],
                            t_xp[:, k, :],
                            start=(k == 0),
                            stop=(k == KC - 1),
                        )
                    if half == 1:
                        # bias-add copy on DVE: the ACT queue is reserved for
                        # exp so proj epilogues never head-of-line block it
                        nc.vector.tensor_scalar(
                            t_dst[:, m, c0 : c0 + 512],
                            p[:],
                            t_b[:, m : m + 1],
                            None,
                            mybir.AluOpType.add,
                        )

                def v_half(tc4, half, t_xp=t_xp, nt=nt):
                    tch = 4 * nt + tc4
                    if half == 0:
                        p = psX.tile([128, 512], F32, tag="aux")
                        live[("v", tc4)] = p
                        ks = range(0, KC // 2)
                    else:
                        p = live.pop(("v", tc4))
                        ks = range(KC // 2, KC)
                    for k in ks:
                        nc.tensor.matmul(
                            p[:, :DLOC],
                            t_xp[:, k, 128 * tc4 : 128 * tc4 + 128],
                            t_wv[:, k, :],
                            start=(k == 0),
                            stop=False,
                        )
                    if half == 1:
                        nc.tensor.matmul(
                            p[:, :DLOC], t_ones[:], t_bvp[:], start=False, stop=True
                        )
                        nc.vector.tensor_copy(
                            out=t_v[:, tch, :, 0:HD],
                            in_=p[:, :DLOC].rearrange("p (h d) -> p h d", h=HLOC),
                        )
                        nc.vector.tensor_copy(
                            out=t_v[:, tch, :, HD], in_=t_vones[:]
                        )

                import functools

                for t_w, t_b, t_dst in ((t_wq, t_bq, t_qt), (t_wk, t_bk, t_kt)):
                    for m in range(2):
                        for half in range(2):
                            groups.append(
                                functools.partial(qk_half, t_w, t_b, t_dst, m, half)
                            )
                for tc4 in range(4):
                    for half in range(2):
                        groups.append(functools.partial(v_half, tc4, half))
                return groups

            # ---- Interleaved schedule: proj(0) runs up front; proj(qt+1)'s
            # eight PSUM-groups are interspersed between attention(qt)'s score
            # block-groups so the PE never drains while ACT works on exp.
            for g in proj(0, t_xp=t_xp0):
                g()
            for g in proj(1):
                g()
            # bulk constants load behind the critical input DMAs
            nc.sync.dma_start(t_mask[:], mask[:].bitcast(F32R))
            nc.sync.dma_start(t_wp[:], wp_r.bitcast(F32R))
            nc.sync.dma_start(t_onesm[:], onesm[:].bitcast(F32R))
            nc.sync.dma_start(t_sums4[:], onesr[:])
            pending = []
            for qt in range(NT):
                stages = {}
                if qt + 2 < NT:
                    pending.extend(proj(qt + 2))
                q0 = 512 * qt
                nblk = 4 * qt + 4
                for hf in range(2):
                    # the pair's two heads run as independent, interleaved
                    # ST->exp->OT chains: while one head's exp is on ACT, the
                    # PE works the sibling head, so neither engine stalls.
                    ngrp = 2 * qt + 2
                    ots = {}
                    exs = {}

                    def emit_stg(pp, g):
                        hp = 64 * pp
                        h = 2 * hf + pp
                        st = psB.tile([128, 2, 512], F32, tag=f"st{pp}")
                        for jj in range(2):
                            j = 2 * g + jj
                            nc.tensor.matmul(
                                st[:, jj, :],
                                t_kt[hp : hp + 64, hf, 128 * j : 128 * j + 128],
                                t_qt[hp : hp + 64, hf, q0 : q0 + 512],
                                start=True,
                                stop=True,
                            )
                        ex = epool.tile([128, 2, 512], F32R, tag=f"ex{pp}")
                        nc.scalar.activation(ex[:], st[:], AF.Exp, scale=float(SCALE))
                        if g >= 2 * qt:
                            m0 = 2 * (g - 2 * qt)
                            off = 128 * m0
                            eng = nc.vector if (pp == 0) else nc.gpsimd
                            eng.tensor_tensor(
                                ex[:, :, off:],
                                ex[:, :, off:],
                                t_mask[:, m0 : m0 + 2, off:512],
                                MUL,
                            )
                        exs[(pp, g)] = ex

                    def emit_otg(pp, g):
                        h = 2 * hf + pp
                        ex = exs.pop((pp, g))
                        for jj in range(2):
                            j = 2 * g + jj
                            off = 128 * (j - 4 * qt) if j >= 4 * qt else 0
                            nc.tensor.matmul(
                                ots[pp][:, off:],
                                t_v[:, j, h, :],
                                ex[:, jj, off:],
                                start=(j == 0),
                                stop=(j == nblk - 1),
                            )

                    for pp in range(2):
                        ots[pp] = psC.tile(
                            [65, 512], F32, tag="ot", name=f"ot_{qt}_{hf}_{pp}"
                        )
                        emit_stg(pp, 0)
                    for g in range(1, ngrp):
                        for pp in range(2):
                            emit_stg(pp, g)
                        for pp in range(2):
                            emit_otg(pp, g - 1)
                        for _ in range(min(2, len(pending))):
                            pending.pop(0)()
                    for pp in range(2):
                        emit_otg(pp, ngrp - 1)
                        # stage unnormalized OT to SBUF (frees the PSUM slot)
                        h = 2 * hf + pp
                        stage = spool.tile([65, 512], F32, tag="stg")
                        nc.vector.tensor_copy(out=stage[:], in_=ots[pp][:])
                        stages[h] = stage
                        # softmax denominator row onto partition 32*h
                        nc.vector.tensor_copy(
                            out=t_sums4[32 * h : 32 * h + 1, :], in_=stage[64:65, :]
                        )
                    if pending:
                        pending.pop(0)()

                # one reciprocal per qt covers all four heads' denominators
                nc.vector.reciprocal(t_rec4[:], t_sums4[:])
                for r in range(4):
                    hfr, pp = divmod(r, 2)
                    hp = 64 * pp
                    bc = psX.tile([128, 512], F32, tag="aux")
                    nc.tensor.matmul(
                        bc[0:64, :],
                        t_onesm[32 * r : 32 * r + 32, r, :],
                        t_rec4[32 * r : 32 * r + 32, :],
                        start=True,
                        stop=True,
                        tile_position=(32 * r, 0),
                    )
                    nc.vector.tensor_tensor(
                        t_ot[hp : hp + 64, hfr, 512 * qt : 512 * qt + 512],
                        bc[0:64, :],
                        stages[r][0:64, :],
                        MUL,
                    )

                # queue the output projection for this qt's q-chunks; it is
                # emitted interleaved into the next qt's attention stream
                def final_qc(qc):
                    ty = wpool.tile([128, D], F32, tag="ty")
                    for n2 in range(2):
                        py = psX.tile([128, 512], F32, tag="aux")
                        for c in range(2):
                            nc.tensor.matmul(
                                py[:],
                                t_ot[:, c, 128 * qc : 128 * qc + 128],
                                t_wp[:, c, 512 * n2 : 512 * n2 + 512],
                                start=(c == 0),
                                stop=(c == 1),
                            )
                        nc.vector.tensor_copy(
                            out=ty[:, 512 * n2 : 512 * n2 + 512], in_=py[:]
                        )
                    nc.gpsimd.dma_start(y[128 * qc : 128 * qc + 128, :], ty[:])

                import functools as _ft

                for qc in range(4 * qt, 4 * qt + 4):
                    pending.append(_ft.partial(final_qc, qc))

            for g in pending:
                g()

    nc.compile()
    return nc


def _get_nc():
    if "nc" not in _CACHE:
        _CACHE["nc"] = _build()
    return _CACHE["nc"]


def _make_in_maps(xp, Wq, bq, Wk, bk, Wv, bv, Wp, bp):
    xp = np.asarray(xp, np.float32)
    Wq, Wk, Wv, Wp = (np.asarray(a, np.float32) for a in (Wq, Wk, Wv, Wp))
    bq, bk, bv, bp = (np.asarray(a, np.float32) for a in (bq, bk, bv, bp))

    maskv = np.zeros((128, 4, 1024), np.float32)
    for m in range(4):
        for p in range(128):
            maskv[p, m, 128 * m + p : 512] = 1.0
            maskv[p, m, 512 + 128 * m + p :] = 1.0
    onesv = np.zeros((32, 128), np.float32)
    onesv[0] = 1.0
    onesmv = np.zeros((128, 4, 64), np.float32)
    for r in range(4):
        onesmv[32 * r, r, :] = 1.0
    onesrv = np.ones((128, 512), np.float32)

    in_maps = []
    for c in range(8):
        b, g = divmod(c, 4)
        s = slice(DLOC * g, DLOC * (g + 1))
        bvpv = np.zeros((32, DLOC), np.float32)
        bvpv[0] = bv[s]
        in_maps.append(
            {
                "xpt": np.ascontiguousarray(xp[b].T),
                "wq": np.ascontiguousarray(Wq[:, s]),
                "wk": np.ascontiguousarray(Wk[:, s]),
                "wv": np.ascontiguousarray(Wv[:, s]),
                "bq": np.ascontiguousarray(bq[s]),
                "bk": np.ascontiguousarray(bk[s]),
                "bvp": bvpv,
                "wp": np.ascontiguousarray(Wp[s, :]),
                "mask": maskv,
                "ones": onesv,
                "onesm": onesmv,
                "onesr": onesrv,
            }
        )

    return in_maps


def _gather(results, bp):
    out = np.zeros((B, T, D), np.float32)
    for c in range(8):
        out[c // 4] += results[c]["y"]
    out += np.asarray(bp, np.float32)[None, None, :]
    return out


def kernel(xp, Wq, bq, Wk, bk, Wv, bv, Wp, bp):
    nc = _get_nc()
    in_maps = _make_in_maps(xp, Wq, bq, Wk, bk, Wv, bv, Wp, bp)
    res = run_bass_kernel_spmd(nc, in_maps, list(range(8)))
    return _gather(res.results, bp)



# revision 8
# speedup vs baseline: 1.1208x; 1.1208x over previous
"""Causal self-attention (B=2, T=2048, D_in=1152, D=1024, H=16) on 8 trn2 cores.

Sharding: 2-way data parallel over batch x 4-way tensor parallel over heads.
Core c handles batch b = c//4 and heads [4g, 4g+4) with g = c%4.

Per-core dataflow (all matmuls in float32r, ~fp32 precision at bf16 speed):
  QT = (Wq_g)^T @ xp[b]^T   -> [256, 2048]   (head dims on partitions)
  KT likewise; V = xp[b] @ Wv_g in natural [T, 256] layout (T on partitions),
  stored with a ones-column per head: Vh~ = [V_h | 1] as [128, 16, 4, 65].
  Scores transposed: ST[k, q] = K Q^T built per 128-row k-block so softmax
  denominators come free: OT~ = Vh~^T @ exp(ST/8) accumulates [65, 512] in
  PSUM where row 64 is the softmax row-sum. exp is unshifted (scores are
  N(0,1) after scaling - safe in fp32). Causal mask = 0/1 multiply after exp
  on the 128-wide triangle band of diagonal blocks only (columns left of the
  band are skipped by the OT matmul's offset; fully-masked blocks skipped).
  Normalize via fast approx reciprocal + rank-1 (K=32 zero-padded) broadcast
  matmul, write into OT_all [256, 2048], then Y_partial = OT_all^T @ Wp_g
  staged to bf16 for the writeback.
Host sums the 4 partial Y per batch and adds bp + bv @ Wp (exact: softmax
rows sum to 1, so the V bias contributes a constant bv @ Wp per output row).
"""

import numpy as np

import concourse.bass as bass
import concourse.mybir as mybir
import concourse.tile as tile
from concourse import bacc
from concourse.bass_utils import run_bass_kernel_spmd

F32 = mybir.dt.float32
F32R = mybir.dt.float32r
BF16 = mybir.dt.bfloat16
AF = mybir.ActivationFunctionType
MUL = mybir.AluOpType.mult

B, T, DIN, D, H = 2, 2048, 1152, 1024, 16
HD = D // H           # 64 head dim
HLOC = 4              # heads per core
DLOC = HLOC * HD      # 256 local model dims
KC = DIN // 128       # 9 contraction chunks for projections
NT = T // 512         # 4 column tiles of 512
QC = T // 128         # 16 row chunks of 128
SCALE = 1.0 / np.sqrt(np.float32(HD))

_CACHE = {}


def _build():
    nc = bacc.Bacc(None)

    xpt = nc.dram_tensor("xpt", [DIN, T], F32, kind="ExternalInput")
    wq = nc.dram_tensor("wq", [DIN, DLOC], F32, kind="ExternalInput")
    wk = nc.dram_tensor("wk", [DIN, DLOC], F32, kind="ExternalInput")
    wv = nc.dram_tensor("wv", [DIN, DLOC], F32, kind="ExternalInput")
    bq = nc.dram_tensor("bq", [DLOC], F32, kind="ExternalInput")
    bk = nc.dram_tensor("bk", [DLOC], F32, kind="ExternalInput")
    wp = nc.dram_tensor("wp", [DLOC, D], F32, kind="ExternalInput")
    band = nc.dram_tensor("band", [128, 128], F32, kind="ExternalInput")
    onesm = nc.dram_tensor("onesm", [128, 4, 64], F32, kind="ExternalInput")
    onesr = nc.dram_tensor("onesr", [128, 512], F32, kind="ExternalInput")
    y = nc.dram_tensor("y", [T, D], BF16, kind="ExternalOutput")

    xpt_r = xpt.rearrange("(ko p) t -> p ko t", p=128)
    wq_r = wq.rearrange("(ko p) d -> p ko d", p=128)
    wk_r = wk.rearrange("(ko p) d -> p ko d", p=128)
    wv_r = wv.rearrange("(ko p) d -> p ko d", p=128)
    wp_r = wp.rearrange("(c p) n -> p c n", p=128)
    bq_r = bq.rearrange("(m p) -> p m", p=128)
    bk_r = bk.rearrange("(m p) -> p m", p=128)

    with tile.TileContext(nc) as tc:
        with (
            tc.tile_pool(name="const", bufs=1) as cpool,
            tc.tile_pool(name="work", bufs=2) as wpool,
            tc.tile_pool(name="exp", bufs=3) as epool,
            tc.tile_pool(name="stg", bufs=5) as spool,
            tc.tile_pool(name="psB", bufs=1, space="PSUM") as psB,
            tc.tile_pool(name="psC", bufs=2, space="PSUM") as psC,
            tc.tile_pool(name="psX", bufs=2, space="PSUM") as psX,
            nc.allow_low_precision(reason="float32r matmul pipeline"),
        ):
            t_wq = cpool.tile([128, KC, DLOC], F32R, tag="t_wq")
            t_wk = cpool.tile([128, KC, DLOC], F32R, tag="t_wk")
            t_wv = cpool.tile([128, KC, DLOC], F32R, tag="t_wv")
            t_wp = cpool.tile([128, 2, D], F32R, tag="t_wp")
            t_band = cpool.tile([128, 128], F32R, tag="t_band")
            t_bq = cpool.tile([128, 2], F32, tag="t_bq")
            t_bk = cpool.tile([128, 2], F32, tag="t_bk")
            t_qt = cpool.tile([128, 2, T], F32R, tag="t_qt")
            t_kt = cpool.tile([128, 2, T], F32R, tag="t_kt")
            t_v = cpool.tile([128, QC, HLOC, HD + 1], F32R, tag="t_v")
            t_ot = cpool.tile([128, 2, T], F32R, tag="t_ot")
            t_onesm = cpool.tile([128, 4, 64], F32R, tag="t_onesm")
            t_rec4 = cpool.tile([128, 512], F32R, tag="t_rec4")
            t_rec4f = cpool.tile([128, 512], F32, tag="t_rec4f")
            t_sums4 = cpool.tile([128, 512], F32, tag="t_sums4")

            t_vones = cpool.tile([128, HLOC], F32R, tag="t_vones")

            # critical-path DMAs first, chunked per contraction block so the
            # k-th projection matmul starts as soon as its chunk lands
            t_xp0 = wpool.tile([128, KC, 512], F32R, tag="t_xp")
            for k in range(KC):
                nc.sync.dma_start(t_wq[:, k, :], wq_r[:, k, :].bitcast(F32R))
                nc.sync.dma_start(
                    t_xp0[:, k, :], xpt_r[:, k, 0:512].bitcast(F32R)
                )
            nc.sync.dma_start(t_wk[:], wk_r.bitcast(F32R))
            nc.sync.dma_start(t_wv[:], wv_r.bitcast(F32R))
            nc.sync.dma_start(t_bq[:], bq_r)
            nc.sync.dma_start(t_bk[:], bk_r)
            nc.sync.dma_start(t_vones[:], onesr[:, 0:HLOC].bitcast(F32R))
            nc.sync.dma_start(t_sums4[:], onesr[:])

            def proj(nt, t_xp=None):
                c0 = 512 * nt
                if t_xp is None:
                    t_xp = wpool.tile([128, KC, 512], F32R, tag="t_xp")
                    nc.sync.dma_start(
                        t_xp[:], xpt_r[:, :, c0 : c0 + 512].bitcast(F32R)
                    )
                groups = []
                live = {}

                def qk_half(t_w, t_b, t_dst, m, half, t_xp=t_xp, c0=c0):
                    if half == 0:
                        p = psX.tile([128, 512], F32, tag="aux")
                        live[(id(t_w), m)] = p
                        ks = range(0, KC // 2)
                    else:
                        p = live.pop((id(t_w), m))
                        ks = range(KC // 2, KC)
                    for k in ks:
                        nc.tensor.matmul(
                            p[:],
                            t_w[:, k, 128 * m : 128 * m + 128],
                            t_xp[:, k, :],
                            start=(k == 0),
                            stop=(k == KC - 1),
                        )
                    if half == 1:
                        # bias-add copy on DVE: the ACT queue is reserved for
                        # exp so proj epilogues never head-of-line block it
                        nc.vector.tensor_scalar(
                            t_dst[:, m, c0 : c0 + 512],
                            p[:],
                            t_b[:, m : m + 1],
                            None,
                            mybir.AluOpType.add,
                        )

                def v_half(tc4, half, t_xp=t_xp, nt=nt):
                    tch = 4 * nt + tc4
                    if half == 0:
                        p = psX.tile([128, 512], F32, tag="aux")
                        live[("v", tc4)] = p
                        ks = range(0, KC // 2)
                    else:
                        p = live.pop(("v", tc4))
                        ks = range(KC // 2, KC)
                    for k in ks:
                        nc.tensor.matmul(
                            p[:, :DLOC],
                            t_xp[:, k, 128 * tc4 : 128 * tc4 + 128],
                            t_wv[:, k, :],
                            start=(k == 0),
                            stop=(k == KC - 1),
                        )
                    if half == 1:
                        nc.vector.tensor_copy(
                            out=t_v[:, tch, :, 0:HD],
                            in_=p[:, :DLOC].rearrange("p (h d) -> p h d", h=HLOC),
                        )
                        nc.vector.tensor_copy(
                            out=t_v[:, tch, :, HD], in_=t_vones[:]
                        )

                import functools

                for t_w, t_b, t_dst in ((t_wq, t_bq, t_qt), (t_wk, t_bk, t_kt)):
                    for m in range(2):
                        for half in range(2):
                            groups.append(
                                functools.partial(qk_half, t_w, t_b, t_dst, m, half)
                            )
                for tc4 in range(4):
                    for half in range(2):
                        groups.append(functools.partial(v_half, tc4, half))
                return groups

            # ---- Interleaved schedule: proj(0) runs up front; proj(qt+1)'s
            # eight PSUM-groups are interspersed between attention(qt)'s score
            # block-groups so the PE never drains while ACT works on exp.
            for g in proj(0, t_xp=t_xp0):
                g()
            for g in proj(1):
                g()
            # bulk constants load behind the critical input DMAs
            nc.sync.dma_start(t_band[:], band[:].bitcast(F32R))
            nc.sync.dma_start(t_wp[:], wp_r.bitcast(F32R))
            nc.sync.dma_start(t_onesm[:], onesm[:].bitcast(F32R))
            pending = []
            for qt in range(NT):
                stages = {}
                if qt + 2 < NT:
                    pending.extend(proj(qt + 2))
                q0 = 512 * qt
                nblk = 4 * qt + 4
                for hf in range(2):
                    # the pair's two heads run as independent, interleaved
                    # ST->exp->OT chains: while one head's exp is on ACT, the
                    # PE works the sibling head, so neither engine stalls.
                    ngrp = 2 * qt + 2
                    ots = {}
                    exs = {}

                    def emit_stg(pp, g):
                        hp = 64 * pp
                        h = 2 * hf + pp
                        st = psB.tile([128, 2, 512], F32, tag=f"st{pp}")
                        for jj in range(2):
                            j = 2 * g + jj
                            nc.tensor.matmul(
                                st[:, jj, :],
                                t_kt[hp : hp + 64, hf, 128 * j : 128 * j + 128],
                                t_qt[hp : hp + 64, hf, q0 : q0 + 512],
                                start=True,
                                stop=True,
                            )
                        ex = epool.tile([128, 2, 512], F32R, tag=f"ex{pp}")
                        nc.scalar.activation(ex[:], st[:], AF.Exp, scale=float(SCALE))
                        if g >= 2 * qt:
                            m0 = 2 * (g - 2 * qt)
                            eng = nc.vector if (pp == 0) else nc.gpsimd
                            for jj in range(2):
                                b0 = 128 * (m0 + jj)
                                eng.tensor_tensor(
                                    ex[:, jj, b0 : b0 + 128],
                                    ex[:, jj, b0 : b0 + 128],
                                    t_band[:],
                                    MUL,
                                )
                        exs[(pp, g)] = ex

                    def emit_otg(pp, g):
                        h = 2 * hf + pp
                        ex = exs.pop((pp, g))
                        for jj in range(2):
                            j = 2 * g + jj
                            off = 128 * (j - 4 * qt) if j >= 4 * qt else 0
                            nc.tensor.matmul(
                                ots[pp][:, off:],
                                t_v[:, j, h, :],
                                ex[:, jj, off:],
                                start=(j == 0),
                                stop=(j == nblk - 1),
                            )

                    for pp in range(2):
                        ots[pp] = psC.tile(
                            [65, 512], F32, tag="ot", name=f"ot_{qt}_{hf}_{pp}"
                        )
                        emit_stg(pp, 0)
                    for g in range(1, ngrp):
                        for pp in range(2):
                            emit_stg(pp, g)
                        for pp in range(2):
                            emit_otg(pp, g - 1)
                        for _ in range(min(2, len(pending))):
                            pending.pop(0)()
                    for pp in range(2):
                        emit_otg(pp, ngrp - 1)
                        # stage unnormalized OT to SBUF (frees the PSUM slot)
                        h = 2 * hf + pp
                        stage = spool.tile([65, 512], F32, tag="stg")
                        nc.vector.tensor_copy(out=stage[:], in_=ots[pp][:])
                        stages[h] = stage
                        # softmax denominator row onto partition 32*h
                        nc.vector.tensor_copy(
                            out=t_sums4[32 * h : 32 * h + 1, :], in_=stage[64:65, :]
                        )
                    if pending:
                        pending.pop(0)()

                # one fast reciprocal per qt covers all four heads' denominators
                # (the copy re-rounds to f32r for the broadcast matmul input)
                nc.vector.reciprocal_approx_fast(t_rec4f[:], t_sums4[:])
                nc.vector.tensor_copy(out=t_rec4[:], in_=t_rec4f[:])
                for r in range(4):
                    hfr, pp = divmod(r, 2)
                    hp = 64 * pp
                    bc = psX.tile([128, 512], F32, tag="aux")
                    nc.tensor.matmul(
                        bc[0:64, :],
                        t_onesm[32 * r : 32 * r + 32, r, :],
                        t_rec4[32 * r : 32 * r + 32, :],
                        start=True,
                        stop=True,
                        tile_position=(32 * r, 0),
                    )
                    nc.vector.tensor_tensor(
                        t_ot[hp : hp + 64, hfr, 512 * qt : 512 * qt + 512],
                        bc[0:64, :],
                        stages[r][0:64, :],
                        MUL,
                    )

                # queue the output projection for this qt's q-chunks; it is
                # emitted interleaved into the next qt's attention stream
                def final_qc(qc):
                    ty = wpool.tile([128, D], BF16, tag="ty")
                    for n2 in range(2):
                        py = psX.tile([128, 512], F32, tag="aux")
                        for c in range(2):
                            nc.tensor.matmul(
                                py[:],
                                t_ot[:, c, 128 * qc : 128 * qc + 128],
                                t_wp[:, c, 512 * n2 : 512 * n2 + 512],
                                start=(c == 0),
                                stop=(c == 1),
                            )
                        nc.vector.tensor_copy(
                            out=ty[:, 512 * n2 : 512 * n2 + 512], in_=py[:]
                        )
                    nc.gpsimd.dma_start(y[128 * qc : 128 * qc + 128, :], ty[:])

                import functools as _ft

                for qc in range(4 * qt, 4 * qt + 4):
                    pending.append(_ft.partial(final_qc, qc))

            for g in pending:
                g()

    nc.compile()
    return nc


def _get_nc():
    if "nc" not in _CACHE:
        _CACHE["nc"] = _build()
    return _CACHE["nc"]


def _make_in_maps(xp, Wq, bq, Wk, bk, Wv, bv, Wp, bp):
    xp = np.asarray(xp, np.float32)
    Wq, Wk, Wv, Wp = (np.asarray(a, np.float32) for a in (Wq, Wk, Wv, Wp))
    bq, bk, bv, bp = (np.asarray(a, np.float32) for a in (bq, bk, bv, bp))

    bandv = np.zeros((128, 128), np.float32)
    for p in range(128):
        bandv[p, p:] = 1.0
    onesmv = np.zeros((128, 4, 64), np.float32)
    for r in range(4):
        onesmv[32 * r, r, :] = 1.0
    onesrv = np.ones((128, 512), np.float32)

    in_maps = []
    for c in range(8):
        b, g = divmod(c, 4)
        s = slice(DLOC * g, DLOC * (g + 1))
        in_maps.append(
            {
                "xpt": np.ascontiguousarray(xp[b].T),
                "wq": np.ascontiguousarray(Wq[:, s]),
                "wk": np.ascontiguousarray(Wk[:, s]),
                "wv": np.ascontiguousarray(Wv[:, s]),
                "bq": np.ascontiguousarray(bq[s]),
                "bk": np.ascontiguousarray(bk[s]),
                "wp": np.ascontiguousarray(Wp[s, :]),
                "band": bandv,
                "onesm": onesmv,
                "onesr": onesrv,
            }
        )

    return in_maps


def _gather(results, bv, Wp, bp):
    out = np.zeros((B, T, D), np.float32)
    for c in range(8):
        out[c // 4] += np.asarray(results[c]["y"], np.float32)
    # softmax rows sum to 1, so the V bias contributes bv @ Wp per row
    bias = (
        np.asarray(bp, np.float64) + np.asarray(bv, np.float64) @ np.asarray(Wp, np.float64)
    ).astype(np.float32)
    out += bias[None, None, :]
    return out


def kernel(xp, Wq, bq, Wk, bk, Wv, bv, Wp, bp):
    nc = _get_nc()
    in_maps = _make_in_maps(xp, Wq, bq, Wk, bk, Wv, bv, Wp, bp)
    res = run_bass_kernel_spmd(nc, in_maps, list(range(8)))
    return _gather(res.results, bv, Wp, bp)


# revision 20
# speedup vs baseline: 1.1387x; 1.0160x over previous
"""Causal self-attention (B=2, T=2048, D_in=1152, D=1024, H=16) on 8 trn2 cores.

Sharding: 2-way data parallel over batch x 4-way tensor parallel over heads.
Core c handles batch b = c//4 and heads [4g, 4g+4) with g = c%4.

Per-core dataflow (all matmuls in float32r, ~fp32 precision at bf16 speed):
  QT = (Wq_g)^T @ xp[b]^T   -> [256, 2048]   (head dims on partitions)
  KT likewise; V = xp[b] @ Wv_g in natural [T, 256] layout (T on partitions),
  stored with a ones-column per head: Vh~ = [V_h | 1] as [128, 16, 4, 65].
  Scores transposed: ST[k, q] = K Q^T built per 128-row k-block so softmax
  denominators come free: OT~ = Vh~^T @ exp(ST/8) accumulates [65, 512] in
  PSUM where row 64 is the softmax row-sum. exp is unshifted (scores are
  N(0,1) after scaling - safe in fp32). Causal mask = 0/1 multiply after exp
  on the 128-wide triangle band of diagonal blocks only (columns left of the
  band are skipped by the OT matmul's offset; fully-masked blocks skipped).
  Normalize via fast approx reciprocal + rank-1 (K=32 zero-padded) broadcast
  matmul, write into OT_all [256, 2048], then Y_partial = OT_all^T @ Wp_g
  staged to bf16 for the writeback.
Host sums the 4 partial Y per batch and adds bp + bv @ Wp (exact: softmax
rows sum to 1, so the V bias contributes a constant bv @ Wp per output row).
"""

import numpy as np

import concourse.bass as bass
import concourse.mybir as mybir
import concourse.tile as tile
from concourse import bacc
from concourse.bass_utils import run_bass_kernel_spmd

F32 = mybir.dt.float32
F32R = mybir.dt.float32r
BF16 = mybir.dt.bfloat16
AF = mybir.ActivationFunctionType
MUL = mybir.AluOpType.mult

B, T, DIN, D, H = 2, 2048, 1152, 1024, 16
HD = D // H           # 64 head dim
HLOC = 4              # heads per core
DLOC = HLOC * HD      # 256 local model dims
KC = DIN // 128       # 9 contraction chunks for projections
NT = T // 512         # 4 column tiles of 512
QC = T // 128         # 16 row chunks of 128
SCALE = 1.0 / np.sqrt(np.float32(HD))

_CACHE = {}


def _build():
    nc = bacc.Bacc(None)

    xpt = nc.dram_tensor("xpt", [DIN, T], F32, kind="ExternalInput")
    wq = nc.dram_tensor("wq", [DIN, DLOC], F32, kind="ExternalInput")
    wk = nc.dram_tensor("wk", [DIN, DLOC], F32, kind="ExternalInput")
    wv = nc.dram_tensor("wv", [DIN, DLOC], F32, kind="ExternalInput")
    bq = nc.dram_tensor("bq", [DLOC], F32, kind="ExternalInput")
    bk = nc.dram_tensor("bk", [DLOC], F32, kind="ExternalInput")
    wp = nc.dram_tensor("wp", [DLOC, D], F32, kind="ExternalInput")
    band = nc.dram_tensor("band", [128, 128], F32, kind="ExternalInput")
    onesm = nc.dram_tensor("onesm", [128, 4, 64], F32, kind="ExternalInput")
    onesr = nc.dram_tensor("onesr", [128, 512], F32, kind="ExternalInput")
    y = nc.dram_tensor("y", [T, D], BF16, kind="ExternalOutput")

    xpt_r = xpt.rearrange("(ko p) t -> p ko t", p=128)
    wq_r = wq.rearrange("(ko p) d -> p ko d", p=128)
    wk_r = wk.rearrange("(ko p) d -> p ko d", p=128)
    wv_r = wv.rearrange("(ko p) d -> p ko d", p=128)
    wp_r = wp.rearrange("(c p) n -> p c n", p=128)
    bq_r = bq.rearrange("(m p) -> p m", p=128)
    bk_r = bk.rearrange("(m p) -> p m", p=128)

    with tile.TileContext(nc) as tc:
        with (
            tc.tile_pool(name="const", bufs=1) as cpool,
            tc.tile_pool(name="work", bufs=2) as wpool,
            tc.tile_pool(name="exp", bufs=3) as epool,
            tc.tile_pool(name="stg", bufs=5) as spool,
            tc.tile_pool(name="psB", bufs=1, space="PSUM") as psB,
            tc.tile_pool(name="psC", bufs=2, space="PSUM") as psC,
            tc.tile_pool(name="psX", bufs=2, space="PSUM") as psX,
            nc.allow_low_precision(reason="float32r matmul pipeline"),
        ):
            t_wq = cpool.tile([128, KC, DLOC], F32R, tag="t_wq")
            t_wk = cpool.tile([128, KC, DLOC], F32R, tag="t_wk")
            t_wv = cpool.tile([128, KC, DLOC], F32R, tag="t_wv")
            t_wp = cpool.tile([128, 2, D], F32R, tag="t_wp")
            t_band = cpool.tile([128, 128], F32R, tag="t_band")
            t_bq = cpool.tile([128, 2], F32, tag="t_bq")
            t_bk = cpool.tile([128, 2], F32, tag="t_bk")
            t_qt = cpool.tile([128, 2, T], F32R, tag="t_qt")
            t_kt = cpool.tile([128, 2, T], F32R, tag="t_kt")
            t_v = cpool.tile([128, QC, HLOC, HD + 1], F32R, tag="t_v")
            t_ot = cpool.tile([128, 2, T], F32R, tag="t_ot")
            t_onesm = cpool.tile([128, 4, 64], F32R, tag="t_onesm")
            t_rec4 = cpool.tile([128, 512], F32R, tag="t_rec4")
            t_rec4f = cpool.tile([128, 512], F32, tag="t_rec4f")
            t_sums4 = cpool.tile([128, 512], F32, tag="t_sums4")

            t_vones = cpool.tile([128, HLOC], F32R, tag="t_vones")

            # critical-path DMAs first, chunked per contraction block so the
            # k-th projection matmul starts as soon as its chunk lands
            t_xp0 = wpool.tile([128, KC, 512], F32R, tag="t_xp")
            for k in range(KC):
                nc.sync.dma_start(t_wq[:, k, :], wq_r[:, k, :].bitcast(F32R))
                nc.sync.dma_start(
                    t_xp0[:, k, :], xpt_r[:, k, 0:512].bitcast(F32R)
                )
            nc.sync.dma_start(t_wk[:], wk_r.bitcast(F32R))
            nc.sync.dma_start(t_wv[:], wv_r.bitcast(F32R))
            nc.sync.dma_start(t_bq[:], bq_r)
            nc.sync.dma_start(t_bk[:], bk_r)
            nc.sync.dma_start(t_vones[:], onesr[:, 0:HLOC].bitcast(F32R))
            nc.sync.dma_start(t_sums4[:], onesr[:])

            def proj(nt, t_xp=None):
                c0 = 512 * nt
                if t_xp is None:
                    t_xp = wpool.tile([128, KC, 512], F32R, tag="t_xp")
                    nc.sync.dma_start(
                        t_xp[:], xpt_r[:, :, c0 : c0 + 512].bitcast(F32R)
                    )
                groups = []
                live = {}

                def qk_half(t_w, t_b, t_dst, m, half, t_xp=t_xp, c0=c0):
                    if half == 0:
                        p = psX.tile([128, 512], F32, tag="aux")
                        live[(id(t_w), m)] = p
                        ks = range(0, KC // 2)
                    else:
                        p = live.pop((id(t_w), m))
                        ks = range(KC // 2, KC)
                    for k in ks:
                        nc.tensor.matmul(
                            p[:],
                            t_w[:, k, 128 * m : 128 * m + 128],
                            t_xp[:, k, :],
                            start=(k == 0),
                            stop=(k == KC - 1),
                        )
                    if half == 1:
                        # bias-add copy on DVE: the ACT queue is reserved for
                        # exp so proj epilogues never head-of-line block it
                        nc.vector.tensor_scalar(
                            t_dst[:, m, c0 : c0 + 512],
                            p[:],
                            t_b[:, m : m + 1],
                            None,
                            mybir.AluOpType.add,
                        )

                def v_half(tc4, half, t_xp=t_xp, nt=nt):
                    tch = 4 * nt + tc4
                    if half == 0:
                        p = psX.tile([128, 512], F32, tag="aux")
                        live[("v", tc4)] = p
                        ks = range(0, KC // 2)
                    else:
                        p = live.pop(("v", tc4))
                        ks = range(KC // 2, KC)
                    for k in ks:
                        nc.tensor.matmul(
                            p[:, :DLOC],
                            t_xp[:, k, 128 * tc4 : 128 * tc4 + 128],
                            t_wv[:, k, :],
                            start=(k == 0),
                            stop=(k == KC - 1),
                        )
                    if half == 1:
                        nc.vector.tensor_copy(
                            out=t_v[:, tch, :, 0:HD],
                            in_=p[:, :DLOC].rearrange("p (h d) -> p h d", h=HLOC),
                        )
                        nc.vector.tensor_copy(
                            out=t_v[:, tch, :, HD], in_=t_vones[:]
                        )

                import functools

                for t_w, t_b, t_dst in ((t_wq, t_bq, t_qt), (t_wk, t_bk, t_kt)):
                    for m in range(2):
                        for half in range(2):
                            groups.append(
                                functools.partial(qk_half, t_w, t_b, t_dst, m, half)
                            )
                for tc4 in range(4):
                    for half in range(2):
                        groups.append(functools.partial(v_half, tc4, half))
                return groups

            # ---- Interleaved schedule: proj(0) runs up front; proj(qt+1)'s
            # eight PSUM-groups are interspersed between attention(qt)'s score
            # block-groups so the PE never drains while ACT works on exp.
            for g in proj(0, t_xp=t_xp0):
                g()
            for g in proj(1):
                g()
            # bulk constants load behind the critical input DMAs
            nc.sync.dma_start(t_band[:], band[:].bitcast(F32R))
            nc.sync.dma_start(t_wp[:], wp_r.bitcast(F32R))
            nc.sync.dma_start(t_onesm[:], onesm[:].bitcast(F32R))
            pending = []
            for qt in range(NT):
                stages = {}
                if qt + 2 < NT:
                    pending.extend(proj(qt + 2))
                q0 = 512 * qt
                nblk = 4 * qt + 4
                for hf in range(2):
                    # the pair's two heads run as independent, interleaved
                    # ST->exp->OT chains: while one head's exp is on ACT, the
                    # PE works the sibling head, so neither engine stalls.
                    ngrp = 2 * qt + 2
                    ots = {}
                    exs = {}

                    def emit_stg(pp, g):
                        hp = 64 * pp
                        h = 2 * hf + pp
                        st = psB.tile([128, 2, 512], F32, tag=f"st{pp}")
                        for jj in range(2):
                            j = 2 * g + jj
                            # diagonal blocks: skip fully-masked columns (the
                            # OT matmul already reads from this offset). qt=0
                            # keeps full width so the exp never reads PSUM
                            # that no score matmul has initialized yet.
                            off = 0  # trim disabled (NaN bisection)
                            nc.tensor.matmul(
                                st[:, jj, off:],
                                t_kt[hp : hp + 64, hf, 128 * j : 128 * j + 128],
                                t_qt[hp : hp + 64, hf, q0 + off : q0 + 512],
                                start=True,
                                stop=True,
                            )
                        ex = epool.tile([128, 2, 512], F32R, tag=f"ex{pp}")
                        nc.scalar.activation(ex[:], st[:], AF.Exp, scale=float(SCALE))
                        if g >= 2 * qt:
                            m0 = 2 * (g - 2 * qt)
                            eng = nc.vector if (pp == 0) else nc.gpsimd
                            for jj in range(2):
                                b0 = 128 * (m0 + jj)
                                eng.tensor_tensor(
                                    ex[:, jj, b0 : b0 + 128],
                                    ex[:, jj, b0 : b0 + 128],
                                    t_band[:],
                                    MUL,
                                )
                        exs[(pp, g)] = ex

                    def emit_otg(pp, g):
                        h = 2 * hf + pp
                        ex = exs.pop((pp, g))
                        for jj in range(2):
                            j = 2 * g + jj
                            off = 128 * (j - 4 * qt) if j >= 4 * qt else 0
                            nc.tensor.matmul(
                                ots[pp][:, off:],
                                t_v[:, j, h, :],
                                ex[:, jj, off:],
                                start=(j == 0),
                                stop=(j == nblk - 1),
                            )

                    for pp in range(2):
                        ots[pp] = psC.tile(
                            [65, 512], F32, tag="ot", name=f"ot_{qt}_{hf}_{pp}"
                        )
                        emit_stg(pp, 0)
                    for g in range(1, ngrp):
                        for pp in range(2):
                            emit_stg(pp, g)
                        if pending:
                            pending.pop(0)()
                        for pp in range(2):
                            emit_otg(pp, g - 1)
                        if pending:
                            pending.pop(0)()
                    for pp in range(2):
                        emit_otg(pp, ngrp - 1)
                        # stage unnormalized OT to SBUF (frees the PSUM slot)
                        h = 2 * hf + pp
                        stage = spool.tile([65, 512], F32, tag="stg")
                        nc.vector.tensor_copy(out=stage[:], in_=ots[pp][:])
                        stages[h] = stage
                        # softmax denominator row onto partition 32*h
                        nc.vector.tensor_copy(
                            out=t_sums4[32 * h : 32 * h + 1, :], in_=stage[64:65, :]
                        )
                    if pending:
                        pending.pop(0)()

                # one fast reciprocal per qt covers all four heads' denominators
                # (the copy re-rounds to f32r for the broadcast matmul input)
                nc.vector.reciprocal_approx_fast(t_rec4f[:], t_sums4[:])
                nc.vector.tensor_copy(out=t_rec4[:], in_=t_rec4f[:])
                for r in range(4):
                    hfr, pp = divmod(r, 2)
                    hp = 64 * pp
                    bc = psX.tile([128, 512], F32, tag="aux")
                    nc.tensor.matmul(
                        bc[0:64, :],
                        t_onesm[32 * r : 32 * r + 32, r, :],
                        t_rec4[32 * r : 32 * r + 32, :],
                        start=True,
                        stop=True,
                        tile_position=(32 * r, 0),
                    )
                    nc.vector.tensor_tensor(
                        t_ot[hp : hp + 64, hfr, 512 * qt : 512 * qt + 512],
                        bc[0:64, :],
                        stages[r][0:64, :],
                        MUL,
                    )

                # queue the output projection for this qt's q-chunks; it is
                # emitted interleaved into the next qt's attention stream
                def final_qc(qc):
                    ty = wpool.tile([128, D], BF16, tag="ty")
                    for n2 in range(2):
                        py = psX.tile([128, 512], F32, tag="aux")
                        for c in range(2):
                            nc.tensor.matmul(
                                py[:],
                                t_ot[:, c, 128 * qc : 128 * qc + 128],
                                t_wp[:, c, 512 * n2 : 512 * n2 + 512],
                                start=(c == 0),
                                stop=(c == 1),
                            )
                        nc.vector.tensor_copy(
                            out=ty[:, 512 * n2 : 512 * n2 + 512], in_=py[:]
                        )
                    nc.sync.dma_start(y[128 * qc : 128 * qc + 128, :], ty[:])

                import functools as _ft

                for qc in range(4 * qt, 4 * qt + 4):
                    pending.append(_ft.partial(final_qc, qc))

            for g in pending:
                g()

    nc.compile()
    return nc


def _get_nc():
    if "nc" not in _CACHE:
        _CACHE["nc"] = _build()
    return _CACHE["nc"]


def _make_in_maps(xp, Wq, bq, Wk, bk, Wv, bv, Wp, bp):
    xp = np.asarray(xp, np.float32)
    Wq, Wk, Wv, Wp = (np.asarray(a, np.float32) for a in (Wq, Wk, Wv, Wp))
    bq, bk, bv, bp = (np.asarray(a, np.float32) for a in (bq, bk, bv, bp))

    bandv = np.zeros((128, 128), np.float32)
    for p in range(128):
        bandv[p, p:] = 1.0
    onesmv = np.zeros((128, 4, 64), np.float32)
    for r in range(4):
        onesmv[32 * r, r, :] = 1.0
    onesrv = np.ones((128, 512), np.float32)

    in_maps = []
    for c in range(8):
        b, g = divmod(c, 4)
        s = slice(DLOC * g, DLOC * (g + 1))
        in_maps.append(
            {
                "xpt": np.ascontiguousarray(xp[b].T),
                "wq": np.ascontiguousarray(Wq[:, s]),
                "wk": np.ascontiguousarray(Wk[:, s]),
                "wv": np.ascontiguousarray(Wv[:, s]),
                "bq": np.ascontiguousarray(bq[s]),
                "bk": np.ascontiguousarray(bk[s]),
                "wp": np.ascontiguousarray(Wp[s, :]),
                "band": bandv,
                "onesm": onesmv,
                "onesr": onesrv,
            }
        )

    return in_maps


def _gather(results, bv, Wp, bp):
    out = np.zeros((B, T, D), np.float32)
    for c in range(8):
        out[c // 4] += np.asarray(results[c]["y"], np.float32)
    # softmax rows sum to 1, so the V bias contributes bv @ Wp per row
    bias = (
        np.asarray(bp, np.float64) + np.asarray(bv, np.float64) @ np.asarray(Wp, np.float64)
    ).astype(np.float32)
    out += bias[None, None, :]
    return out


def kernel(xp, Wq, bq, Wk, bk, Wv, bv, Wp, bp):
    nc = _get_nc()
    in_maps = _make_in_maps(xp, Wq, bq, Wk, bk, Wv, bv, Wp, bp)
    res = run_bass_kernel_spmd(nc, in_maps, list(range(8)))
    return _gather(res.results, bv, Wp, bp)
